# revision 1
# baseline (speedup 1.0000x reference)
"""Trainium2 Bass kernel for nn_Attention_module_52166672777937.

Data-parallel over batch across 8 NeuronCores (4 sequences per core).

Algorithmic restructuring (numerically validated against the reference):
the module only consumes the attention output at the LAST valid position
of each sequence (take_along_axis with lengths-1), and attention is
causal, so only ONE query row per sequence matters.  Consequences:

  * q is computed for a single position per sequence.
  * K is never materialized: scores = (qblk.T @ Wk) @ x.T, using
    associativity of the K projection with the score contraction.
  * softmax runs over [H=8, L] scores per sequence (no L x L matrix).
  * ctx = softmax(scores) @ V needs V = x @ Wv.T for all positions -- the
    dominant matmul, kept on TensorE at fp32r full rate.

Device layout: x is built in transposed [E, L] layout directly via a
one-hot matmul gather (onehot[c, l] = (data[l] == c), x.T = emb.T @
onehot + pe.T), which feeds both the score matmul and the V projection
without any transposes of large tensors.
"""

import math
import sys

import ml_dtypes
import numpy as np

sys.path.insert(0, "/opt/trn_rl_repo")

import concourse.bacc as bacc
import concourse.bass as bass
import concourse.mybir as mybir
import concourse.tile as tile
from concourse.bass_utils import run_bass_kernel_spmd

dt = mybir.dt
AF = mybir.ActivationFunctionType
ALU = mybir.AluOpType
PSUM = bass.MemorySpace.PSUM

N_CORES = 8
B, L = 32, 1000
LP = 1024                 # padded sequence length (2 x 512 column tiles)
TW = 512                  # column-tile width (max fp32 moving operand / PSUM bank)
NT = LP // TW             # column tiles per sequence
BPC = B // N_CORES        # sequences per core
NCH = 256                 # vocabulary
E = 512                   # embedding dim
D = 512                   # d_model
NH, DH = 8, 64            # heads
HS = 512                  # pred hidden size
NOUT = 8
NEG = -1.0e30
SCALE = 1.0 / math.sqrt(DH)


def _build():
    nc = bacc.Bacc(
        "TRN2", target_bir_lowering=False, debug=False, num_devices=N_CORES
    )

    f32 = dt.float32
    f32r = dt.float32r
    bf16 = dt.bfloat16
    # --- packed inputs (few wide DMAs instead of many narrow ones) ------
    # bf16s: data+idxlast row; emb; wqT|wk|wvT; peT; id8b
    d_drow = nc.dram_tensor("drow", [1, BPC * LP + BPC], bf16,
                            kind="ExternalInput")
    d_emb = nc.dram_tensor("emb", [NCH, E], bf16, kind="ExternalInput")
    d_wb = nc.dram_tensor("wb", [E, 3 * D], bf16, kind="ExternalInput")
    d_peT = nc.dram_tensor("peT", [E, LP], bf16, kind="ExternalInput")
    d_id8b = nc.dram_tensor("id8b", [NH, NH], bf16, kind="ExternalInput")
    # f32r: w1T|w2T ; ones8
    d_wr = nc.dram_tensor("wr", [D, HS + NOUT], f32r, kind="ExternalInput")
    d_ones8 = nc.dram_tensor("ones8", [NH, 1], f32r, kind="ExternalInput")
    # f32: pelastT|hmask (512-row); bq|b1|cvals (128-row); plast|b2|iota (8-row)
    d_fA = nc.dram_tensor("fA", [D, BPC + NH], f32, kind="ExternalInput")
    d_fB = nc.dram_tensor("fB", [128, 42], f32, kind="ExternalInput")
    d_fC = nc.dram_tensor("fC", [NH, BPC + 1 + LP], f32, kind="ExternalInput")
    d_out = nc.dram_tensor("out", [1, BPC], f32, kind="ExternalOutput")

    with tile.TileContext(nc) as tc:
        with (
            tc.tile_pool(name="const", bufs=1) as cp,
            tc.tile_pool(name="work", bufs=2) as wp,
            tc.tile_pool(name="psx", bufs=2, space=PSUM) as psx,
            tc.tile_pool(name="psv", bufs=2, space=PSUM) as psv,
            tc.tile_pool(name="pss", bufs=2, space=PSUM) as pss,
            tc.tile_pool(name="psc", bufs=1, space=PSUM) as psc,
            tc.tile_pool(name="psd", bufs=1, space=PSUM) as psd,
        ):
            # ---------------- constant loads (packed) -------------------
            # issued in order of first use so compute starts early:
            # fB(cvals) -> db0 -> emb -> wq -> wk -> peT -> wv -> ...
            fB_sb = cp.tile([128, 42], f32, name="fB", tag="fB")
            nc.sync.dma_start(out=fB_sb[:], in_=d_fB[:])
            bq_sb = fB_sb[:, 0:4]
            b1_sb = fB_sb[:, 4:8]
            cvals_sb = fB_sb[:, 8:10]
            maskT_sb = fB_sb[:, 10:42]
            ones128_sb = cp.tile([128, 1], bf16, name="ones128",
                                 tag="ones128")
            nc.vector.memset(ones128_sb[:], 1.0)
            dbb = []
            for b in range(BPC):
                t = cp.tile([128, LP], bf16, name=f"db{b}", tag=f"db{b}")
                dbb.append(t)
            nc.sync.dma_start(
                out=dbb[0][:],
                in_=d_drow[:, 0:LP].to_broadcast((128, LP)),
            )
            embp_sb = cp.tile([128, 2, E], bf16, name="embp", tag="embp")
            nc.sync.dma_start(
                out=embp_sb[:],
                in_=d_emb[:].rearrange("(c p) n -> p c n", p=128),
            )
            emb_sb = [embp_sb[:, c, :] for c in range(2)]
            wqp_sb = cp.tile([128, 4, D], bf16, name="wqp", tag="wqp")
            nc.sync.dma_start(
                out=wqp_sb[:],
                in_=d_wb[:, 0:D].rearrange("(c p) n -> p c n", p=128),
            )
            wqT_sb = [wqp_sb[:, e, :] for e in range(4)]
            wkp_sb = cp.tile([128, 4, D], bf16, name="wkp", tag="wkp")
            nc.sync.dma_start(
                out=wkp_sb[:],
                in_=d_wb[:, D:2 * D].rearrange("(c p) n -> p c n", p=128),
            )
            wk_sb = [wkp_sb[:, c, :] for c in range(4)]
            peTp_sb = cp.tile([128, 4, LP], bf16, name="peTp", tag="peTp")
            nc.sync.dma_start(
                out=peTp_sb[:],
                in_=d_peT[:].rearrange("(c p) n -> p c n", p=128),
            )
            peT_sb = [peTp_sb[:, e, :] for e in range(4)]
            wvp_sb = cp.tile([128, 4, D], bf16, name="wvp", tag="wvp")
            nc.sync.dma_start(
                out=wvp_sb[:],
                in_=d_wb[:, 2 * D:3 * D].rearrange("(c p) n -> p c n", p=128),
            )
            wvT_sb = [wvp_sb[:, e, :] for e in range(4)]
            idxb_sb = cp.tile([128, BPC], bf16, name="idxb", tag="idxb")
            nc.sync.dma_start(
                out=idxb_sb[:],
                in_=d_drow[:, BPC * LP:].to_broadcast((128, BPC)),
            )
            fA_sb = cp.tile([128, 4, BPC + NH], f32, name="fA", tag="fA")
            nc.sync.dma_start(
                out=fA_sb[:], in_=d_fA[:].rearrange("(c p) n -> p c n", p=128)
            )
            pelT_sb = [fA_sb[:, m, 0:BPC] for m in range(4)]
            hmask_sb = [fA_sb[:, m, BPC:BPC + NH] for m in range(4)]
            fC_sb = cp.tile([NH, BPC + 1 + LP], f32, name="fC", tag="fC")
            nc.sync.dma_start(out=fC_sb[:], in_=d_fC[:])
            plast_sb = fC_sb[:, 0:BPC]
            b2_sb = fC_sb[:, BPC:BPC + 1]
            iota_sb = fC_sb[:, BPC + 1:]
            id8b_sb = cp.tile([NH, NH], bf16, name="id8b", tag="id8b")
            nc.sync.dma_start(out=id8b_sb[:], in_=d_id8b[:])
            for b in range(1, BPC):
                nc.sync.dma_start(
                    out=dbb[b][:],
                    in_=d_drow[:, b * LP:(b + 1) * LP].to_broadcast(
                        (128, LP)),
                )
            drow_sb = cp.tile([1, BPC * LP + BPC], bf16, name="drow",
                              tag="drow")
            nc.sync.dma_start(out=drow_sb[:], in_=d_drow[:])
            wr_sb = cp.tile([128, 4, HS + NOUT], f32r, name="wr", tag="wr")
            nc.sync.dma_start(
                out=wr_sb[:], in_=d_wr[:].rearrange("(c p) n -> p c n", p=128)
            )
            w1T_sb = [wr_sb[:, m, 0:HS] for m in range(4)]
            w2T_sb = [wr_sb[:, m, HS:HS + NOUT] for m in range(4)]
            ones8_sb = cp.tile([NH, 1], f32r, name="ones8", tag="ones8")
            nc.sync.dma_start(out=ones8_sb[:], in_=d_ones8[:])

            madd_sb = [None] * BPC

            def emit_gather(b, t):
                # one-hot + x.T tile for (sequence b, column tile t)
                oh = []
                for c in range(2):
                    o = wp.tile([128, TW], bf16, name=f"oh{b}_{t}_{c}",
                                tag="oh", bufs=6)
                    nc.vector.tensor_scalar(
                        o[:], dbb[b][:, t * TW:(t + 1) * TW],
                        cvals_sb[:, c:c + 1], None, ALU.is_equal,
                    )
                    oh.append(o)
                xT = []
                for e in range(4):
                    p = psx.tile([128, TW], f32, name=f"xtp{b}_{t}_{e}",
                                 tag="xtp")
                    for c in range(2):
                        nc.tensor.matmul(
                            p[:], (emb_sb[c][:, e * 128:(e + 1) * 128]),
                            (oh[c][:]), start=(c == 0), stop=(c == 1),
                        )
                    x = wp.tile([128, TW], bf16, name=f"xT{b}_{t}_{e}",
                                tag=f"xT{e}", bufs=3)
                    nc.vector.tensor_tensor(
                        x[:], p[:], peT_sb[e][:, t * TW:(t + 1) * TW],
                        ALU.add,
                    )
                    xT.append(x)
                return xT

            # first tile's gather depends only on the earliest DMAs; emit it
            # ahead of the serial q-prep chain so the PE queue head has work
            xT_first = emit_gather(0, 0)

            # ---------------- x_last gather -> q ------------------------
            ohl = []
            for c in range(2):
                t = cp.tile([128, BPC], bf16, name=f"ohl{c}", tag=f"ohl{c}")
                nc.vector.tensor_scalar(
                    t[:], idxb_sb[:], cvals_sb[:, c:c + 1], None, ALU.is_equal
                )
                ohl.append(t)
            # x_last.T [E, BPC] = emb.T @ onehot_last + pe_last.T
            xlast_sb = []
            for e in range(4):
                p = psx.tile([128, BPC], f32, name=f"xlp{e}", tag="xtp")
                for c in range(2):
                    nc.tensor.matmul(
                        p[:], (emb_sb[c][:, e * 128:(e + 1) * 128]),
                        (ohl[c][:]), start=(c == 0), stop=(c == 1),
                    )
                t = cp.tile([128, BPC], bf16, name=f"xlast{e}", tag=f"xlast{e}")
                nc.vector.tensor_tensor(t[:], p[:], pelT_sb[e][:], ALU.add)
                xlast_sb.append(t)
            # q.T [D, BPC] = Wq @ x_last.T + bq
            qT_sb = []
            for d in range(4):
                p = psv.tile([128, BPC], f32, name=f"qp{d}", tag="vp")
                for e in range(4):
                    nc.tensor.matmul(
                        p[:], (wqT_sb[e][:, d * 128:(d + 1) * 128]),
                        (xlast_sb[e][:]), start=(e == 0), stop=(e == 3),
                    )
                t = cp.tile([128, BPC], f32, name=f"qT{d}", tag=f"qT{d}")
                nc.vector.tensor_scalar(t[:], p[:], bq_sb[:, d:d + 1], None,
                                        ALU.add)
                qT_sb.append(t)

            # ---------------- main loop over sequences ------------------
            out_sb = cp.tile([1, BPC], f32, name="out_sb", tag="out_sb")
            ctxT_sb = [cp.tile([128, BPC], f32r, name=f"ctxT{m}", tag=f"ctxT{m}")
                       for m in range(4)]
            for b in range(BPC):
                # --- per-sequence qkvec = qblk.T @ Wk (K never formed) --
                qblk = []
                for d in range(4):
                    t = cp.tile([128, NH], bf16, name=f"qblk{b}_{d}",
                                tag=f"qblk{b}_{d}")
                    nc.vector.tensor_scalar(
                        t[:], hmask_sb[d][:], qT_sb[d][:, b:b + 1], None,
                        ALU.mult,
                    )
                    qblk.append(t)
                qkvp = pss.tile([NH, E], f32, name=f"qkvp{b}", tag="sp")
                for d in range(4):
                    nc.tensor.matmul(
                        qkvp[:], (qblk[d][:]), (wk_sb[d][:]),
                        start=(d == 0), stop=(d == 3),
                    )
                qkv_sb = wp.tile([NH, E], bf16, name=f"qkv{b}", tag="qkv",
                                 bufs=2)
                nc.vector.tensor_copy(qkv_sb[:], qkvp[:])
                qkvT = []
                for e in range(4):
                    tp = pss.tile([128, NH], bf16, name=f"qkvTp{b}_{e}",
                                  tag="sp")
                    nc.tensor.transpose(
                        tp[:], qkv_sb[:, e * 128:(e + 1) * 128], id8b_sb[:]
                    )
                    t = cp.tile([128, NH], bf16, name=f"qkvT{b}_{e}",
                                tag=f"qkvT{b}_{e}")
                    nc.vector.tensor_copy(t[:], tp[:])
                    qkvT.append(t)

                # --- attention over the sequence ------------------------
                ctxp = psc.tile([NH, D], f32, name=f"ctx{b}", tag="cp")
                dnp = psd.tile([1, NH], f32, name=f"dn{b}", tag="dn")
                for t in range(NT):
                    xT = xT_first if (b == 0 and t == 0) else emit_gather(b, t)
                    # per l-chunk: transposed scores [l, h], exp with the
                    # causal mask as the per-partition ACT bias, V, ctx
                    for lc in range(4):
                        slp = pss.tile([128, NH], f32,
                                       name=f"sl{b}_{t}_{lc}", tag="sp")
                        for e in range(4):
                            nc.tensor.matmul(
                                slp[:],
                                (xT[e][:, lc * 128:(lc + 1) * 128]),
                                (qkvT[e][:]),
                                start=(e == 0), stop=(e == 3),
                            )
                        aT = wp.tile([128, NH], bf16, name=f"aT{b}_{t}_{lc}",
                                     tag="aT", bufs=8)
                        mcol = b * 8 + t * 4 + lc
                        nc.scalar.activation(
                            aT[:], slp[:], AF.Exp, scale=SCALE,
                            bias=maskT_sb[:, mcol:mcol + 1],
                        )
                        nc.tensor.matmul(
                            dnp[:], ones128_sb[:], aT[:],
                            start=(t == 0 and lc == 0),
                            stop=(t == NT - 1 and lc == 3),
                        )
                        vp = psv.tile([128, D], f32, name=f"vp{b}_{t}_{lc}",
                                      tag="vp")
                        for e in range(4):
                            nc.tensor.matmul(
                                vp[:],
                                (xT[e][:, lc * 128:(lc + 1) * 128]),
                                (wvT_sb[e][:]),
                                start=(e == 0), stop=(e == 3),
                            )
                        v = wp.tile([128, D], bf16, name=f"v{b}_{t}_{lc}",
                                    tag="v", bufs=4)
                        if lc % 2 == 0:
                            nc.vector.tensor_copy(v[:], vp[:])
                        else:
                            nc.scalar.copy(v[:], vp[:])
                        nc.tensor.matmul(
                            ctxp[:], (aT[:]), (v[:]),
                            start=(t == 0 and lc == 0),
                            stop=(t == NT - 1 and lc == 3),
                        )
                # normalize ctx rows by the masked softmax denominator;
                # den is [1, 8] (summed over partitions via ones-matmul),
                # transpose to [8, 1] with a K=1 matmul (identity scalar
                # borrowed from the iota column whose value is 1.0)
                dn_sb = wp.tile([1, NH], f32, name=f"dns{b}", tag="dns",
                                bufs=2)
                nc.vector.tensor_copy(dn_sb[:], dnp[:])
                dTp = pss.tile([NH, 1], f32, name=f"dTp{b}", tag="sp")
                nc.tensor.transpose(
                    dTp[:], dn_sb[:], fC_sb[0:1, BPC + 2:BPC + 3]
                )
                dsum = wp.tile([NH, 1], f32, name=f"dsum{b}", tag="dsum",
                               bufs=2)
                nc.vector.tensor_copy(dsum[:], dTp[:])
                rec = wp.tile([NH, 1], f32, name=f"rec{b}", tag="rec", bufs=2)
                nc.vector.reciprocal(rec[:], dsum[:])
                ctx_sb = wp.tile([NH, D], bf16, name=f"ctxs{b}", tag="ctxs",
                                 bufs=2)
                nc.scalar.activation(ctx_sb[:], ctxp[:], AF.Copy,
                                     scale=rec[:])
                # extract block-diagonal -> ctx.T [D, BPC] column b
                for m in range(4):
                    tp = pss.tile([128, NH], bf16, name=f"ctp{b}_{m}", tag="sp")
                    nc.tensor.transpose(
                        tp[:], ctx_sb[:, m * 128:(m + 1) * 128], id8b_sb[:]
                    )
                    scr = wp.tile([128, NH], f32, name=f"scr{b}_{m}",
                                  tag="scr", bufs=2)
                    nc.vector.tensor_tensor(scr[:], tp[:], hmask_sb[m][:],
                                            ALU.mult)
                    with nc.allow_low_precision("fp32 accum, fp32r round"):
                        nc.vector.tensor_reduce(
                            ctxT_sb[m][:, b:b + 1], scr[:],
                            mybir.AxisListType.X, ALU.add,
                        )

            # ---------------- prediction head ---------------------------
            hT_sb = []
            for hc in range(4):
                p = psv.tile([128, BPC], f32, name=f"hp{hc}", tag="vp")
                for m in range(4):
                    nc.tensor.matmul(
                        p[:], (w1T_sb[m][:, hc * 128:(hc + 1) * 128]),
                        (ctxT_sb[m][:]), start=(m == 0), stop=(m == 3),
                    )
                t1 = wp.tile([128, BPC], f32, name=f"t1_{hc}", tag="t1",
                             bufs=2)
                nc.vector.tensor_scalar(t1[:], p[:], b1_sb[:, hc:hc + 1],
                                        None, ALU.add)
                ht = cp.tile([128, BPC], f32r, name=f"hT{hc}", tag=f"hT{hc}")
                nc.vector.scalar_tensor_tensor(
                    ht[:], t1[:], 0.01, t1[:], ALU.mult, ALU.max
                )
                hT_sb.append(ht)
            r2p = pss.tile([NOUT, BPC], f32, name="r2p", tag="sp")
            for hc in range(4):
                nc.tensor.matmul(
                    r2p[:], (w2T_sb[hc][:]), (hT_sb[hc][:]),
                    start=(hc == 0), stop=(hc == 3),
                )
            r_sb = cp.tile([NOUT, BPC], f32r, name="r_sb", tag="r_sb")
            nc.vector.tensor_scalar(r_sb[:], r2p[:], b2_sb[:], 0.0,
                                    ALU.add, ALU.max)
            mp = pss.tile([1, BPC], f32, name="mp", tag="sp")
            nc.tensor.matmul(mp[:], (ones8_sb[:]), (r_sb[:]))
            mt = cp.tile([1, BPC], f32, name="mt", tag="mt")
            nc.vector.tensor_scalar(mt[:], mp[:], 1.0 / NOUT, None, ALU.mult)
            nc.vector.scalar_tensor_tensor(
                out_sb[:], mt[:], 0.01, mt[:], ALU.mult, ALU.max
            )
            nc.sync.dma_start(out=d_out[:], in_=out_sb[:])

    nc.compile()
    return nc


_CACHE = {}


def _get_module():
    if "nc" not in _CACHE:
        _CACHE["nc"] = _build()
    return _CACHE["nc"]


def _pos_encoding():
    pos = np.arange(L, dtype=np.float32)[:, None]
    div = np.exp(
        np.arange(0, D, 2, dtype=np.float32) * (-math.log(10000.0) / D)
    )
    pe = np.zeros((L, D), np.float32)
    pe[:, 0::2] = np.sin(pos * div)
    pe[:, 1::2] = np.cos(pos * div)
    return pe


def make_in_maps(data, lengths, emb, Wq, bq, Wk, bk, Wv, bv, W1, b1, W2, b2):
    # the kernel folds the K-projection into the score contraction; a
    # nonzero bk would add a per-head constant q.bk_h to the scores, which
    # this build omits (bk is zero for this module).
    assert float(np.abs(np.asarray(bk)).max()) == 0.0
    # V eviction is a plain copy; nonzero bv would need a bias add there.
    assert float(np.abs(np.asarray(bv)).max()) == 0.0

    pe = _pos_encoding()                       # [L, D]
    peT = np.zeros((E, LP), np.float32)
    peT[:, :L] = pe.T

    dpad = np.zeros((B, LP), np.int64)
    dpad[:, :L] = data
    data_f32 = dpad.astype(np.float32)

    p = (np.asarray(lengths).astype(np.int64) - 1)          # [B]
    idxl = np.asarray(data)[np.arange(B), p].astype(np.float32)
    pelT = pe[p].astype(np.float32).T                       # [D, B]

    bfl = ml_dtypes.bfloat16
    wb = np.concatenate(
        [np.asarray(Wq).T, np.asarray(Wk), np.asarray(Wv).T], axis=1
    ).astype(bfl)                                            # [512, 1536]
    wr = np.concatenate(
        [np.asarray(W1).T, np.asarray(W2).T], axis=1
    ).astype(np.float32)                                     # [512, 520]
    fB_head = np.concatenate(
        [np.asarray(bq).reshape(4, 128).T,
         np.asarray(b1).reshape(4, 128).T,
         np.arange(256, dtype=np.float32).reshape(2, 128).T], axis=1
    ).astype(np.float32)                                     # [128, 10]
    iota8 = np.broadcast_to(np.arange(LP, dtype=np.float32), (NH, LP))
    shared = {
        "emb": np.ascontiguousarray(emb, dtype=bfl),
        "wb": np.ascontiguousarray(wb),
        "wr": np.ascontiguousarray(wr),
        "peT": peT.astype(bfl),
        "id8b": np.eye(NH, dtype=bfl),
        "ones8": np.ones((NH, 1), np.float32),
    }
    in_maps = []
    for c in range(N_CORES):
        sl = slice(c * BPC, (c + 1) * BPC)
        m = dict(shared)
        l_of = (np.arange(8)[None, :] * 128
                + np.arange(128)[:, None])                   # [128, 8]
        mT = np.where(
            l_of[:, None, :] > p[sl][None, :, None], -1.0e30, 0.0
        ).reshape(128, BPC * 8).astype(np.float32)
        m["fB"] = np.ascontiguousarray(
            np.concatenate([fB_head, mT], axis=1))           # [128, 42]
        m["drow"] = np.concatenate(
            [data_f32[sl].reshape(-1), idxl[sl]]
        ).reshape(1, -1).astype(bfl)
        m["fA"] = np.ascontiguousarray(np.concatenate(
            [pelT[:, sl], np.repeat(np.eye(NH, dtype=np.float32), DH, axis=0)],
            axis=1)).astype(np.float32)                      # [512, 12]
        m["fC"] = np.ascontiguousarray(np.concatenate(
            [np.broadcast_to(p[sl].astype(np.float32), (NH, BPC)),
             np.asarray(b2).reshape(NOUT, 1).astype(np.float32),
             iota8], axis=1)).astype(np.float32)             # [8, 1029]
        in_maps.append(m)
    return in_maps


def kernel(data, lengths, emb, Wq, bq, Wk, bk, Wv, bv, W1, b1, W2, b2):
    nc = _get_module()
    in_maps = make_in_maps(
        np.asarray(data), np.asarray(lengths), emb, Wq, bq, Wk, bk, Wv, bv,
        W1, b1, W2, b2,
    )
    res = run_bass_kernel_spmd(nc, in_maps, list(range(N_CORES)))
    out = np.concatenate(
        [res.results[c]["out"].reshape(BPC) for c in range(N_CORES)]
    )
    return out.astype(np.float32)



# revision 13
# speedup vs baseline: 1.8377x; 1.8377x over previous
"""Trainium2 Bass kernel for nn_Attention_module_52166672777937.

Data-parallel over batch across 8 NeuronCores (4 sequences per core),
with the 4 sequences x 8 heads STACKED on 32 partitions (s=(b,h)) so
every matmul serves all four sequences at once.

Algorithmic restructuring (validated vs the reference in bf16):
  * Only the LAST query row of causal attention is consumed, so scores
    are [32, L] per core, not [B,H,L,L].
  * x = emb[data] + pe is NEVER materialized.  Scores decompose as
      scores[s,l] = s_emb[s, data[l]] + (qk_s . peT[:,l]) + mask
    where s_emb = qkv @ emb.T is a per-head 256-entry lookup table and
    the data lookup is a one-hot matmul.
  * ctx = attn @ x @ Wv.T similarly decomposes:
      y = attn @ x = (attn @ onehot.T) @ emb + attn @ pe.
  * softmax normalization is folded into the attn transposes by using
    diag(1/denominator) as the transpose "identity" matrix.
  * q = Wq(emb[last] + pe[last]) + bq folds to  (emb@Wq.T).T @ onehot_last
    + qpe  with qpe computed host-side from lengths.
"""

import math
import sys

import ml_dtypes
import numpy as np

sys.path.insert(0, "/opt/trn_rl_repo")

import concourse.bacc as bacc
import concourse.bass as bass
import concourse.mybir as mybir
import concourse.tile as tile
from concourse.bass_utils import run_bass_kernel_spmd

dt = mybir.dt
AF = mybir.ActivationFunctionType
ALU = mybir.AluOpType
AX = mybir.AxisListType
PSUM = bass.MemorySpace.PSUM

N_CORES = 8
B, L = 32, 1000
LP = 1024
BPC = B // N_CORES        # 4 sequences per core
NS = BPC * 8              # 32 stacked (seq, head) rows
NCH = 256
E = 512
D = 512
NH, DH = 8, 64
HS = 512
NOUT = 8
SCALE = 1.0 / math.sqrt(DH)
NLC = LP // 128           # 8 position chunks


def _build():
    nc = bacc.Bacc(
        "TRN2", target_bir_lowering=False, debug=False, num_devices=N_CORES
    )

    f32 = dt.float32
    b16 = dt.bfloat16

    # ---- DRAM inputs -------------------------------------------------
    # f32 [128, 55]: qpe | b1 | b2(8 rows) | cvals | dT
    d_f32 = nc.dram_tensor("f32", [128, 55], f32, kind="ExternalInput")
    d_wqe = nc.dram_tensor("wqe", [NCH, D], b16, kind="ExternalInput")
    d_wk = nc.dram_tensor("wk", [D, E], b16, kind="ExternalInput")
    d_embT = nc.dram_tensor("embT", [E, NCH], b16, kind="ExternalInput")
    d_peT = nc.dram_tensor("peT", [E, LP], b16, kind="ExternalInput")
    # [32, 1604]: maskneg(4r) | E4(4r) | hm32 | Rsel | id32
    d_m32 = nc.dram_tensor("m32", [NS, 1604], b16, kind="ExternalInput")
    # [128, 288]: iotaC | hmask4
    d_sa = nc.dram_tensor("sa", [128, 288], b16, kind="ExternalInput")
    d_pe = nc.dram_tensor("pe", [LP, E], b16, kind="ExternalInput")
    d_emb = nc.dram_tensor("emb", [NCH, E], b16, kind="ExternalInput")
    d_wvT = nc.dram_tensor("wvT", [E, D], b16, kind="ExternalInput")
    d_w1T = nc.dram_tensor("w1T", [D, HS], b16, kind="ExternalInput")
    d_w2T = nc.dram_tensor("w2T", [HS, NOUT], b16, kind="ExternalInput")
    d_drow = nc.dram_tensor("drow", [1, BPC * LP + BPC], b16,
                            kind="ExternalInput")
    d_out = nc.dram_tensor("out", [1, BPC], f32, kind="ExternalOutput")

    with tile.TileContext(nc) as tc:
        with (
            tc.tile_pool(name="const", bufs=1) as cp,
            tc.tile_pool(name="work", bufs=2) as wp,
            tc.tile_pool(name="psbig", bufs=2, space=PSUM) as psb,
            tc.tile_pool(name="pstr", bufs=2, space=PSUM) as pst,
            tc.tile_pool(name="psw", bufs=2, space=PSUM) as psw,
            tc.tile_pool(name="psq", bufs=2, space=PSUM) as psq,
        ):
            # ------------- DMA: critical path on sync queue -----------
            f32_sb = cp.tile([128, 55], f32, name="f32", tag="f32")
            nc.sync.dma_start(out=f32_sb[:], in_=d_f32[:])
            qpe_sb = f32_sb[:, 0:16]     # [128, 4d x 4b]
            b1_sb = f32_sb[:, 16:20]
            b2_sb = f32_sb[0:NOUT, 20:21]
            cvals = f32_sb[:, 21:23]
            dTall = f32_sb[:, 23:55]
            idxb_sb = cp.tile([128, BPC], b16, name="idxb", tag="idxb")
            nc.sync.dma_start(
                out=idxb_sb[:],
                in_=d_drow[:, BPC * LP:].to_broadcast((128, BPC)),
            )
            wqe_sb = cp.tile([128, 2, D], b16, name="wqe", tag="wqe")
            nc.sync.dma_start(
                out=wqe_sb[:], in_=d_wqe[:].rearrange("(c p) n -> p c n", p=128)
            )
            wk_sb = cp.tile([128, 4, E], b16, name="wk", tag="wk")
            nc.sync.dma_start(
                out=wk_sb[:], in_=d_wk[:].rearrange("(c p) n -> p c n", p=128)
            )
            embT_sb = cp.tile([128, 4, NCH], b16, name="embT", tag="embT")
            nc.sync.dma_start(
                out=embT_sb[:],
                in_=d_embT[:].rearrange("(c p) n -> p c n", p=128),
            )
            peT_sb = cp.tile([128, 4, LP], b16, name="peT", tag="peT")
            nc.sync.dma_start(
                out=peT_sb[:], in_=d_peT[:].rearrange("(c p) n -> p c n", p=128)
            )
            m32_sb = cp.tile([NS, 1604], b16, name="m32", tag="m32")
            nc.sync.dma_start(out=m32_sb[:], in_=d_m32[:])
            maskneg = m32_sb[0:BPC, 0:LP]
            E4_sb = m32_sb[0:BPC, LP:LP + NS]
            hm32_sb = m32_sb[:, 1056:1568]
            Rsel_sb = m32_sb[:, 1568:1572]
            id32_sb = m32_sb[:, 1572:1604]

            # ------------- DMA: bulk on the (idle) pool queue ---------
            dbb = []
            for b in range(BPC):
                t = cp.tile([128, LP], b16, name=f"db{b}", tag=f"db{b}")
                nc.scalar.dma_start(
                    out=t[:],
                    in_=d_drow[:, b * LP:(b + 1) * LP].to_broadcast((128, LP)),
                )
                dbb.append(t)
            sa_sb = cp.tile([128, 288], b16, name="sa", tag="sa")
            nc.scalar.dma_start(out=sa_sb[:], in_=d_sa[:])
            iotaC = sa_sb[:, 0:256]
            hmask4 = sa_sb[:, 256:288]
            pe_sb = cp.tile([128, NLC, E], b16, name="pe", tag="pe")
            nc.scalar.dma_start(
                out=pe_sb[:], in_=d_pe[:].rearrange("(c p) n -> p c n", p=128)
            )
            emb_sb = cp.tile([128, 2, E], b16, name="emb", tag="emb")
            nc.scalar.dma_start(
                out=emb_sb[:], in_=d_emb[:].rearrange("(c p) n -> p c n", p=128)
            )
            wvT_sb = cp.tile([128, 4, D], b16, name="wvT", tag="wvT")
            nc.scalar.dma_start(
                out=wvT_sb[:], in_=d_wvT[:].rearrange("(c p) n -> p c n", p=128)
            )
            w1T_sb = cp.tile([128, 4, HS], b16, name="w1T", tag="w1T")
            nc.scalar.dma_start(
                out=w1T_sb[:], in_=d_w1T[:].rearrange("(c p) n -> p c n", p=128)
            )
            w2T_sb = cp.tile([128, 4, NOUT], b16, name="w2T", tag="w2T")
            nc.scalar.dma_start(
                out=w2T_sb[:], in_=d_w2T[:].rearrange("(c p) n -> p c n", p=128)
            )
            ones8_sb = cp.tile([NOUT, 1], b16, name="ones8", tag="ones8")
            nc.gpsimd.memset(ones8_sb[:], 1.0)
            s_embm = cp.tile([128, 2, BPC, NS], b16, name="s_embm",
                             tag="s_embm")
            nc.gpsimd.memset(s_embm[:], 0.0)

            # ------------- q prep (emitted first: heads the queues) ---
            ohl = cp.tile([128, 2, BPC], b16, name="ohl", tag="ohl")
            for c in range(2):
                nc.vector.tensor_scalar(
                    ohl[:, c, :], idxb_sb[:], cvals[:, c:c + 1], None,
                    ALU.is_equal,
                )
            # qT [d, b] = WqE.T @ ohl + qpe   (WqE = emb @ Wq.T)
            qT_sb = cp.tile([128, 4, BPC], f32, name="qT", tag="qT")
            for dch in range(4):
                p = psq.tile([128, BPC], f32, name=f"qp{dch}", tag="qp")
                for c in range(2):
                    nc.tensor.matmul(
                        p[:], wqe_sb[:, c, dch * 128:(dch + 1) * 128],
                        ohl[:, c, :], start=(c == 0), stop=(c == 1),
                    )
                nc.vector.tensor_tensor(
                    qT_sb[:, dch, :], p[:],
                    qpe_sb[:, dch * BPC:(dch + 1) * BPC], ALU.add,
                )
            # qblk [d, s] = q[d, b] * hmask[d, h]
            qblk = cp.tile([128, 4, NS], b16, name="qblk", tag="qblk")
            for dch in range(4):
                for b in range(BPC):
                    nc.vector.tensor_scalar(
                        qblk[:, dch, b * NH:(b + 1) * NH],
                        hmask4[:, dch * NH:(dch + 1) * NH],
                        qT_sb[:, dch, b:b + 1], None, ALU.mult,
                    )
            # qkvT [e, s] = Wk.T @ qblk, scaled by 1/sqrt(dh)
            qkvT = cp.tile([128, 4, NS], b16, name="qkvT", tag="qkvT")
            for ech in range(4):
                p = psq.tile([128, NS], f32, name=f"qkp{ech}", tag="qp")
                for dch in range(4):
                    nc.tensor.matmul(
                        p[:], wk_sb[:, dch, ech * 128:(ech + 1) * 128],
                        qblk[:, dch, :], start=(dch == 0), stop=(dch == 3),
                    )
                nc.vector.tensor_scalar(
                    qkvT[:, ech, :], p[:], SCALE, None, ALU.mult,
                )
            # s_embT [c, s] = embT.T @ qkvT, evicted per-seq masked
            for c in range(2):
                p = psq.tile([128, NS], f32, name=f"sep{c}", tag="qp")
                for ech in range(4):
                    nc.tensor.matmul(
                        p[:], embT_sb[:, ech, c * 128:(c + 1) * 128],
                        qkvT[:, ech, :], start=(ech == 0), stop=(ech == 3),
                    )
                for b in range(BPC):
                    nc.scalar.copy(
                        s_embm[:, c, b, b * NH:(b + 1) * NH],
                        p[:, b * NH:(b + 1) * NH],
                    )

            # ------------- one-hots [c, l] on DVE ---------------------
            oh_sb = cp.tile([128, 2, BPC, LP], b16, name="oh", tag="oh")
            for b in range(BPC):
                for c in range(2):
                    nc.vector.tensor_scalar(
                        oh_sb[:, c, b, :], dbb[b][:], cvals[:, c:c + 1], None,
                        ALU.is_equal,
                    )

            # ------------- scores [32, L] + exp -----------------------
            attn = cp.tile([NS, LP], b16, name="attn", tag="attn")
            for hl in range(2):
                sc = psb.tile([NS, 512], f32, name=f"sc{hl}", tag="big")
                lo, hi = hl * 512, (hl + 1) * 512
                for ech in range(4):
                    nc.tensor.matmul(
                        sc[:], qkvT[:, ech, :], peT_sb[:, ech, lo:hi],
                        start=(ech == 0), stop=False,
                    )
                nc.tensor.matmul(
                    sc[:], E4_sb, maskneg[:, lo:hi], start=False, stop=False,
                )
                for b in range(BPC):
                    for c in range(2):
                        nc.tensor.matmul(
                            sc[:], s_embm[:, c, b, :], oh_sb[:, c, b, lo:hi],
                            start=False, stop=(b == BPC - 1 and c == 1),
                        )
                nc.scalar.activation(attn[:, lo:hi], sc[:], AF.Exp)

            # ------------- one-hots [l, c] (first half) ---------------
            ohT_sb = cp.tile([128, BPC, NLC, NCH], b16, name="ohT", tag="ohT")
            for b in range(2):
                for lc in range(NLC):
                    nc.vector.tensor_scalar(
                        ohT_sb[:, b, lc, :], iotaC,
                        dTall[:, b * NLC + lc:b * NLC + lc + 1], None,
                        ALU.is_equal,
                    )

            # ------------- softmax denominator -> diag(1/dn) ----------
            dn = wp.tile([NS, 1], f32, name="dn", tag="dn")
            nc.vector.tensor_reduce(dn[:], attn[:], AX.X, ALU.add)
            rec = wp.tile([NS, 1], f32, name="rec", tag="rec")
            nc.vector.reciprocal(rec[:], dn[:])

            for b in range(2, BPC):
                for lc in range(NLC):
                    nc.vector.tensor_scalar(
                        ohT_sb[:, b, lc, :], iotaC,
                        dTall[:, b * NLC + lc:b * NLC + lc + 1], None,
                        ALU.is_equal,
                    )

            # ------------- aT [l, s] = normalized attn.T --------------
            # transpose identity = diag(1/dn) folds the softmax division
            # w accumulation + y_pe interleave per chunk right behind it
            aT = cp.tile([128, NLC, NS], b16, name="aT", tag="aT")
            for lc in range(NLC):
                tp = pst.tile([128, NS], b16, name=f"tp{lc}", tag="tr",
                              bufs=2)
                nc.tensor.transpose(
                    tp[:], attn[:, lc * 128:(lc + 1) * 128], id32_sb
                )
                if lc % 2 == 0:
                    nc.vector.tensor_copy(aT[:, lc, :], tp[:])
                else:
                    nc.scalar.copy(aT[:, lc, :], tp[:])
            wps = [
                psw.tile([NH, 2, NCH], f32, name=f"wp{i}", tag="wp")
                for i in range(2)
            ]
            yp = psb.tile([NS, E], f32, name="yp", tag="big")
            for lc in range(NLC):
                for b in range(BPC):
                    nc.tensor.matmul(
                        wps[b // 2][:, b % 2, :],
                        aT[:, lc, b * NH:(b + 1) * NH],
                        ohT_sb[:, b, lc, :],
                        start=(lc == 0), stop=(lc == NLC - 1),
                    )
                nc.tensor.matmul(
                    yp[:], aT[:, lc, :], pe_sb[:, lc, :],
                    start=(lc == 0), stop=False,
                )

            # ------------- wT_all [c, s] via transposes ---------------
            w_sb = cp.tile([NH, BPC, NCH], b16, name="w_sb", tag="w_sb")
            for b in range(BPC):
                nc.scalar.copy(w_sb[:, b, :], wps[b // 2][:, b % 2, :])
            wT_all = cp.tile([128, 2, NS], b16, name="wT_all", tag="wT_all")
            for b in range(BPC):
                for c in range(2):
                    tp = pst.tile([128, NH], b16, name=f"wt{b}_{c}", tag="tr",
                                  bufs=2)
                    nc.tensor.transpose(
                        tp[:], w_sb[:, b, c * 128:(c + 1) * 128],
                        id32_sb[0:NH, 0:NH],
                    )
                    if c == 0:
                        nc.vector.tensor_copy(
                            wT_all[:, c, b * NH:(b + 1) * NH], tp[:])
                    else:
                        nc.scalar.copy(
                            wT_all[:, c, b * NH:(b + 1) * NH], tp[:])

            # ------------- y += wT.T @ emb ; yT ; z = y @ Wv.T --------
            for c in range(2):
                nc.tensor.matmul(
                    yp[:], wT_all[:, c, :], emb_sb[:, c, :],
                    start=False, stop=(c == 1),
                )
            y_sb = wp.tile([NS, E], b16, name="y_sb", tag="y_sb")
            nc.scalar.activation(y_sb[:], yp[:], AF.Copy, scale=rec[:])
            yT = cp.tile([128, 4, NS], b16, name="yT", tag="yT")
            for ech in range(4):
                tp = pst.tile([128, NS], b16, name=f"yt{ech}", tag="tr",
                              bufs=2)
                nc.tensor.transpose(
                    tp[:], y_sb[:, ech * 128:(ech + 1) * 128], id32_sb
                )
                if ech % 2 == 0:
                    nc.vector.tensor_copy(yT[:, ech, :], tp[:])
                else:
                    nc.scalar.copy(yT[:, ech, :], tp[:])
            zp = psb.tile([NS, D], f32, name="zp", tag="big")
            for ech in range(4):
                nc.tensor.matmul(
                    zp[:], yT[:, ech, :], wvT_sb[:, ech, :],
                    start=(ech == 0), stop=(ech == 3),
                )
            zm = wp.tile([NS, D], b16, name="zm", tag="zm")
            nc.vector.tensor_tensor(zm[:], zp[:], hm32_sb, ALU.mult)
            ctxT = cp.tile([128, 4, BPC], b16, name="ctxT", tag="ctxT")
            for m in range(4):
                p = pst.tile([128, BPC], f32, name=f"cx{m}", tag="tr",
                             bufs=2)
                nc.tensor.matmul(
                    p[:], zm[:, m * 128:(m + 1) * 128], Rsel_sb,
                )
                if m % 2 == 0:
                    nc.vector.tensor_copy(ctxT[:, m, :], p[:])
                else:
                    nc.scalar.copy(ctxT[:, m, :], p[:])

            # ------------- prediction head ----------------------------
            hT = cp.tile([128, 4, BPC], b16, name="hT", tag="hT")
            for hc in range(4):
                p = psq.tile([128, BPC], f32, name=f"hp{hc}", tag="qp")
                for m in range(4):
                    nc.tensor.matmul(
                        p[:], w1T_sb[:, m, hc * 128:(hc + 1) * 128],
                        ctxT[:, m, :], start=(m == 0), stop=(m == 3),
                    )
                t1 = wp.tile([128, BPC], f32, name=f"t1_{hc}", tag="t1",
                             bufs=2)
                nc.vector.tensor_scalar(t1[:], p[:], b1_sb[:, hc:hc + 1],
                                        None, ALU.add)
                nc.vector.scalar_tensor_tensor(
                    hT[:, hc, :], t1[:], 0.01, t1[:], ALU.mult, ALU.max
                )
            r2p = pst.tile([NOUT, BPC], f32, name="r2p", tag="tr", bufs=2)
            for hc in range(4):
                nc.tensor.matmul(
                    r2p[:], w2T_sb[:, hc, :], hT[:, hc, :],
                    start=(hc == 0), stop=(hc == 3),
                )
            r_sb = wp.tile([NOUT, BPC], b16, name="r_sb", tag="r_sb")
            nc.vector.tensor_scalar(r_sb[:], r2p[:], b2_sb, 0.0,
                                    ALU.add, ALU.max)
            mp = pst.tile([1, BPC], f32, name="mp", tag="tr", bufs=2)
            nc.tensor.matmul(mp[:], ones8_sb[:], r_sb[:])
            mt = wp.tile([1, BPC], f32, name="mt", tag="mt")
            nc.vector.tensor_scalar(mt[:], mp[:], 1.0 / NOUT, None, ALU.mult)
            out_sb = cp.tile([1, BPC], f32, name="out_sb", tag="out_sb")
            nc.vector.scalar_tensor_tensor(
                out_sb[:], mt[:], 0.01, mt[:], ALU.mult, ALU.max
            )
            nc.sync.dma_start(out=d_out[:], in_=out_sb[:])

    nc.compile()
    return nc


_CACHE = {}


def _get_module():
    if "nc" not in _CACHE:
        _CACHE["nc"] = _build()
    return _CACHE["nc"]


def _pos_encoding():
    pos = np.arange(LP, dtype=np.float32)[:, None]
    div = np.exp(
        np.arange(0, D, 2, dtype=np.float32) * (-math.log(10000.0) / D)
    )
    pe = np.zeros((LP, D), np.float32)
    pe[:, 0::2] = np.sin(pos * div)
    pe[:, 1::2] = np.cos(pos * div)
    return pe


def make_in_maps(data, lengths, emb, Wq, bq, Wk, bk, Wv, bv, W1, b1, W2, b2):
    # the kernel folds the K-projection into the score lookup; a nonzero
    # bk would add a per-head constant to the scores (bk is zero here).
    assert float(np.abs(np.asarray(bk)).max()) == 0.0
    assert float(np.abs(np.asarray(bv)).max()) == 0.0

    b16 = ml_dtypes.bfloat16
    emb = np.asarray(emb, np.float32)
    Wq, Wk, Wv = (np.asarray(a, np.float32) for a in (Wq, Wk, Wv))
    W1, W2 = np.asarray(W1, np.float32), np.asarray(W2, np.float32)
    pe = _pos_encoding()                          # [LP, D]
    data = np.asarray(data)
    lengths = np.asarray(lengths)
    p = (lengths.astype(np.int64) - 1)

    WqE = emb @ Wq.T                              # [256, 512]
    qpe_full = Wq @ pe[p].T + np.asarray(bq, np.float32)[:, None]  # [D, B]
    hmask = np.repeat(np.eye(NH, dtype=np.float32), DH, axis=0)    # [D, 8]

    cvals = (np.arange(2)[None, :] * 128
             + np.arange(128)[:, None]).astype(np.float32)
    iotaC = np.broadcast_to(np.arange(NCH, dtype=np.float32), (128, NCH))
    hmask4 = hmask.reshape(4, 128, NH).transpose(1, 0, 2).reshape(128, 32)

    E4 = np.zeros((BPC, NS), np.float32)
    for b in range(BPC):
        E4[b, b * NH:(b + 1) * NH] = 1.0
    hm32 = np.zeros((NS, D), np.float32)
    for b in range(BPC):
        for h in range(NH):
            hm32[b * NH + h, h * DH:(h + 1) * DH] = 1.0
    Rsel = np.zeros((NS, BPC), np.float32)
    for b in range(BPC):
        Rsel[b * NH:(b + 1) * NH, b] = 1.0
    id32 = np.eye(NS, dtype=np.float32)

    dpad = np.zeros((B, LP), np.int64)
    dpad[:, :L] = data
    idxl = data[np.arange(B), p].astype(np.float32)

    shared = {
        "wqe": np.ascontiguousarray(WqE, dtype=b16),
        "wk": np.ascontiguousarray(Wk, dtype=b16),
        "embT": np.ascontiguousarray(emb.T, dtype=b16),
        "peT": np.ascontiguousarray(pe.T, dtype=b16),
        "pe": np.ascontiguousarray(pe, dtype=b16),
        "emb": np.ascontiguousarray(emb, dtype=b16),
        "wvT": np.ascontiguousarray(Wv.T, dtype=b16),
        "w1T": np.ascontiguousarray(W1.T, dtype=b16),
        "w2T": np.ascontiguousarray(W2.T, dtype=b16),
    }
    sa = np.concatenate([iotaC, hmask4], axis=1)
    shared["sa"] = np.ascontiguousarray(sa, dtype=b16)

    in_maps = []
    for core in range(N_CORES):
        sl = slice(core * BPC, (core + 1) * BPC)
        m = dict(shared)
        dT = np.zeros((128, 32), np.float32)
        for b in range(BPC):
            for lc in range(NLC):
                dT[:, b * NLC + lc] = dpad[sl][b, lc * 128:(lc + 1) * 128]

        maskneg = np.where(
            np.arange(LP)[None, :] > p[sl][:, None], -30000.0, 0.0
        ).astype(np.float32)                       # [4, LP]
        m32 = np.zeros((NS, 1604), np.float32)
        m32[0:BPC, 0:LP] = maskneg
        m32[0:BPC, LP:LP + NS] = E4
        m32[:, 1056:1568] = hm32
        m32[:, 1568:1572] = Rsel
        m32[:, 1572:1604] = id32
        m["m32"] = np.ascontiguousarray(m32, dtype=b16)

        fb = np.zeros((128, 55), np.float32)
        fb[:, 0:16] = qpe_full[:, sl].reshape(4, 128, BPC).transpose(
            1, 0, 2).reshape(128, 16)
        fb[:, 16:20] = np.asarray(b1, np.float32).reshape(4, 128).T
        fb[0:NOUT, 20] = np.asarray(b2, np.float32)
        fb[:, 21:23] = cvals
        fb[:, 23:55] = dT
        m["f32"] = np.ascontiguousarray(fb)

        m["drow"] = np.concatenate(
            [dpad[sl].astype(np.float32).reshape(-1), idxl[sl]]
        ).reshape(1, -1).astype(b16)
        in_maps.append(m)
    return in_maps


def kernel(data, lengths, emb, Wq, bq, Wk, bk, Wv, bv, W1, b1, W2, b2):
    nc = _get_module()
    in_maps = make_in_maps(
        np.asarray(data), np.asarray(lengths), emb, Wq, bq, Wk, bk, Wv, bv,
        W1, b1, W2, b2,
    )
    res = run_bass_kernel_spmd(nc, in_maps, list(range(N_CORES)))
    out = np.concatenate(
        [res.results[c]["out"].reshape(BPC) for c in range(N_CORES)]
    )
    return out.astype(np.float32)
